# revision 34
# baseline (speedup 1.0000x reference)
"""Trainium2 Bass kernel for CustomMultiHeadAttention (B=2, L=2048, D=512, H=8).

Sharding: 8 cores = 2 batches x 4 head-pairs. Each core computes, for its
batch b and its 2 heads, the partial output (O_h @ Wo_h summed over its
heads), transposed: poutT [512, 2048]. Host sums the 4 partials per batch,
transposes, and adds bo.

Device-side math per core (all masking folded into matmul contractions):
  Qh = (q[b]*qm) @ WqT_cols + qm*bq_cols          (masked q rows -> exactly 0)
  Kh = k[b] @ WkT_cols/8 + bk_cols/8
  E[k,q] = Kh.Qh + (kb[k]-c)*qm[q] + c            via 2 extra contraction rows
           (kb = -1e4 for masked keys, c = ln(1/2048))
    -> unmasked q: E = s + kb  (masked keys underflow to 0 in exp)
    -> masked q:   E = c       (exp = 1/2048 uniform; denom = 1)
  PT = exp(E)   [k, q] layout
  outT = [Vp | 1]^T @ PT   (Vp = coef * Vh; ones column yields denom row)
  O = outT[0:64] / outT[64]
  poutT[d, q] += Wo[:, d] . O2[:, q]   (both heads packed, contraction 128)

Perf notes vs the first version:
  - x and weights travel as bf16 (halves input DMA), x in [128,512] blocks
    chunk-major so the first projection matmul starts ~2us in.
  - PE warm-up matmuls at t=0 ride the DVFS ramp (0.65->2.4GHz after ~3us
    of continuous busy).
  - Output projection packs both heads into contraction-128 matmuls.
  - reciprocal_approx_fast (~5x faster than reciprocal); norm + finals are
    chunked at 512 cols so the tail pipelines instead of serializing.
  - poutT returns as bf16 (halves output DMA).
"""

import math
import os

os.environ.setdefault("MYCRO_LOCAL_CACHE", "1")

import numpy as np

import concourse.bass as bass
import concourse.tile as tile
from concourse import bacc
from concourse import mybir
from concourse.bass_utils import run_bass_kernel_spmd
from concourse.masks import make_identity

B = 2
L = 2048
DM = 512
H = 8
DH = 64
NCORES = 8
HPC = 2           # heads per core
DH2 = HPC * DH    # 128
NKT = L // 128    # 16 k tiles
QH = 1024         # q chunk for attention phase
NQH = L // QH     # 2
C_LN = -math.log(L)
NEG = -10000.0

F32 = mybir.dt.float32
F32R = mybir.dt.float32r
BF16 = mybir.dt.bfloat16
F8 = mybir.dt.float8e4

ATT_DT = BF16     # exp output (PT), V'
QK_DT = F32R      # QE/KE on-chip operands (full-rate, ~tf32 rounding)
X_DT = BF16       # x / w wire + projection matmul dtype
NKT2 = NKT // 2   # key tile PAIRS for DoubleRow PV

TRACE = False
LAST_RESULT = None

AUX_QM, AUX_KBMC, AUX_ONES, AUX_CLN = 0, 1, 2, 3


def build_nc(with_bias: bool):
    nc = bacc.Bacc(None, target_bir_lowering=False)

    xqT_d = nc.declare_dram_parameter("xqT", [DM, L], X_DT, isOutput=False)
    xkT_d = nc.declare_dram_parameter("xkT", [DM, L], X_DT, isOutput=False)
    xvT_d = nc.declare_dram_parameter("xvT", [DM, L], X_DT, isOutput=False)
    # weights pre-arranged host-side to the SBUF layout [p, t, m] so the DMA
    # is one contiguous 1KB-per-partition transfer (the on-the-fly rearrange
    # generated 256B packets and crawled at ~37 GB/s).
    wqs_d = nc.declare_dram_parameter("wqs", [128, 4 * DH2], X_DT, isOutput=False)
    wks_d = nc.declare_dram_parameter("wks", [128, 4 * DH2], X_DT, isOutput=False)
    wvs_d = nc.declare_dram_parameter("wvs", [128, 4 * DH2], X_DT, isOutput=False)
    if with_bias:
        wbias_d = nc.declare_dram_parameter(
            "wbias", [1, 4 * DH2], X_DT, isOutput=False
        )
    wos_d = nc.declare_dram_parameter("wos", [DH2, DM], BF16, isOutput=False)
    aux_d = nc.declare_dram_parameter("aux", [4, L], QK_DT, isOutput=False)
    coef_d = nc.declare_dram_parameter("coef", [128, NKT], F32, isOutput=False)
    # poutT stored as 16 contiguous [128, 512] blocks, index (qh*2+c2)*4+dt4;
    # host reassembles. Contiguous blocks DMA at full rate (strided rows of
    # the [DM, L] layout only reached ~78 GB/s).
    pout_d = nc.declare_dram_parameter("poutT", [16, 128, 512], BF16, isOutput=True)

    with tile.TileContext(nc) as tc:
        with (
            tc.tile_pool(name="const", bufs=1) as const,
            tc.tile_pool(name="qek", bufs=1) as qek,
            tc.tile_pool(name="xin", bufs=1) as xin,
            tc.tile_pool(name="vtmp", bufs=1) as vtmp,
            tc.tile_pool(name="ptp", bufs=2) as ptp,
            tc.tile_pool(name="sbB", bufs=1) as sbB,
            tc.tile_pool(name="ps", bufs=1, space="PSUM") as ps,
        ):
            # ---- PE warm-up: ride the DVFS ramp while input DMAs land ----
            wrm = const.tile([128, 512], X_DT)
            nc.vector.memset(wrm, 1.0)
            for w in range(7):
                pw = ps.tile([128, 512], F32, tag="small", bufs=2, name=f"warm{w}")
                nc.tensor.matmul(
                    pw, lhsT=wrm[:, 0:128], rhs=wrm, start=True, stop=True
                )

            # ---- constants: all small tensors stream before the x blocks ----
            ident = const.tile([128, 128], F32)
            make_identity(nc, ident)
            wk_sb = const.tile([128, 4, DH2], X_DT)
            nc.sync.dma_start(out=wk_sb, in_=wks_d[:, :])
            wq_sb = const.tile([128, 4, DH2], X_DT)
            nc.scalar.dma_start(out=wq_sb, in_=wqs_d[:, :])
            wv_sb = const.tile([128, 4, DH2], X_DT)
            nc.scalar.dma_start(out=wv_sb, in_=wvs_d[:, :])
            wo_sb = const.tile([DH2, DM], BF16)
            nc.scalar.dma_start(out=wo_sb, in_=wos_d[:, :])
            coef_sb = const.tile([128, NKT], F32)
            nc.scalar.dma_start(out=coef_sb, in_=coef_d[:, :])
            if with_bias:
                wb_sb = const.tile([1, 4 * DH2], X_DT)
                nc.sync.dma_start(out=wb_sb, in_=wbias_d[:, :])
                qm_sb = const.tile([1, L], X_DT)
                ones_sb = const.tile([1, L], X_DT)
                nc.vector.memset(ones_sb, 1.0)

            # ---- x input blocks: [128, 512], chunk-major so chunk 0 of a
            # projection is computable after 4 small DMAs. Blocks alternate
            # between the two hardware DMA queues (SP + ACT engines) for
            # ~2x aggregate HBM read bandwidth; the ACT engine is idle
            # during the projection phase so its queue is free.
            def x_blocks(pname, xdram, split=True):
                blocks = []
                for ch in range(4):
                    col = []
                    for t in range(4):
                        xt = xin.tile(
                            [128, 512], X_DT, tag="xin", bufs=48,
                            name=f"x{pname}{t}_{ch}",
                        )
                        eng = nc.scalar if (split and t >= 2) else nc.sync
                        eng.dma_start(
                            out=xt,
                            in_=xdram[
                                t * 128 : (t + 1) * 128,
                                ch * 512 : (ch + 1) * 512,
                            ],
                        )
                        col.append(xt)
                    blocks.append(col)
                return blocks

            # ---- persistent per-head operands ----
            QE = [qek.tile([66, L], QK_DT, name=f"QE{h}") for h in range(HPC)]
            KE = [qek.tile([66, L], QK_DT, name=f"KE{h}") for h in range(HPC)]
            Vp = [
                qek.tile([128, NKT, DH + 1], ATT_DT, name=f"Vp{h}") for h in range(HPC)
            ]
            # mask/bias rows of the extended operands (DMA direct from host aux)
            for h in range(HPC):
                nc.sync.dma_start(
                    out=QE[h][64:65, :], in_=aux_d[AUX_QM : AUX_QM + 1, :]
                )
                nc.sync.dma_start(
                    out=QE[h][65:66, :], in_=aux_d[AUX_ONES : AUX_ONES + 1, :]
                )
                nc.scalar.dma_start(
                    out=KE[h][64:65, :], in_=aux_d[AUX_KBMC : AUX_KBMC + 1, :]
                )
                nc.scalar.dma_start(
                    out=KE[h][65:66, :], in_=aux_d[AUX_CLN : AUX_CLN + 1, :]
                )
                nc.vector.memset(Vp[h][:, :, DH : DH + 1], 1.0)

            # k streams first (b1 needs all of K), then the q chunks that
            # feed qh=0 (cols 0-1023), then the rest of q during attention.
            # All x blocks ride the SP queue; the ACT queue only carries the
            # small consts (x blocks there would stall the exp stream).
            xk_b = x_blocks("k", xkT_d, split=False)
            xq_b = x_blocks("q", xqT_d, split=False)
            if with_bias:
                nc.sync.dma_start(out=qm_sb, in_=aux_d[AUX_QM : AUX_QM + 1, :])

            def emit_proj(pname, xb, w_sb, brow, brhs, evict, chunks=(0, 1, 2, 3)):
                for ch in chunks:
                    sl = slice(ch * 512, (ch + 1) * 512)
                    psp = ps.tile([128, 512], F32, tag="small", bufs=2, name="psp")
                    for t in range(4):
                        nc.tensor.matmul(
                            psp,
                            lhsT=w_sb[:, t, :],
                            rhs=xb[ch][t],
                            start=(t == 0),
                            stop=(t == 3 and not with_bias),
                        )
                    if with_bias:
                        nc.tensor.matmul(
                            psp,
                            lhsT=wb_sb[0:1, brow * DH2 : (brow + 1) * DH2],
                            rhs=brhs[0:1, sl],
                            start=False,
                            stop=True,
                        )
                    evict(psp, sl)

            def evict_qk(dst):
                def _e(psp, sl):
                    for h in range(HPC):
                        nc.vector.tensor_copy(
                            out=dst[h][0:DH, sl], in_=psp[h * DH : (h + 1) * DH, :]
                        )

                return _e

            def b1_step(qh, h, pt, kt):
                st = ps.tile([128, QH], F32, tag="st", bufs=2, name="st")
                for c2 in range(QH // 512):
                    nc.tensor.matmul(
                        st[:, c2 * 512 : (c2 + 1) * 512],
                        lhsT=KE[h][0:66, kt * 128 : (kt + 1) * 128],
                        rhs=QE[h][
                            0:66, qh * QH + c2 * 512 : qh * QH + (c2 + 1) * 512
                        ],
                        start=True,
                        stop=True,
                    )
                nc.scalar.activation(
                    out=pt[:, kt, :], in_=st, func=mybir.ActivationFunctionType.Exp
                )

            def b1_steps(qh, h, pt):
                for kt in range(NKT):
                    yield lambda kt=kt: b1_step(qh, h, pt, kt)

            def b2_steps(qh, h, pt, outp):
                for kt in range(NKT):
                    def _s(kt=kt):
                        for c2 in range(QH // 512):
                            nc.tensor.matmul(
                                outp[:, c2 * 512 : (c2 + 1) * 512],
                                lhsT=Vp[h][:, kt, :],
                                rhs=pt[:, kt, c2 * 512 : (c2 + 1) * 512],
                                start=(kt == 0),
                                stop=(kt == NKT - 1),
                            )
                    yield _s

            def interleave(*gens):
                gens = [iter(g) for g in gens if g is not None]
                while gens:
                    nxt = []
                    for g in gens:
                        try:
                            next(g)()
                        except StopIteration:
                            continue
                        nxt.append(g)
                    gens = nxt

            from concourse.alu_op_type import AluOpType

            def emit_norm_chunk(h, outp, nrm2, c2):
                sl = slice(c2 * 512, (c2 + 1) * 512)
                den = sbB.tile([1, 512], F32, tag="den", bufs=2, name="den")
                nc.vector.tensor_copy(out=den, in_=outp[DH : DH + 1, sl])
                rcp = sbB.tile([1, 512], F32, tag="rcp", bufs=2, name="rcp")
                nc.vector.reciprocal_approx_fast(out=rcp, in_=den)
                rbc = sbB.tile([DH, 512], F32, tag="rbc", bufs=2, name="rbc")
                nc.gpsimd.partition_broadcast(rbc, rcp[0:1, :], channels=DH)
                # nrm2 = (outp * 1.0) * rbc  — fused PSUM read + scale
                nc.vector.scalar_tensor_tensor(
                    out=nrm2[h * DH : (h + 1) * DH, sl],
                    in0=outp[0:DH, sl],
                    scalar=1.0,
                    in1=rbc,
                    op0=AluOpType.mult,
                    op1=AluOpType.mult,
                )

            def emit_norm(h, outp, nrm2):
                for c2 in range(QH // 512):
                    emit_norm_chunk(h, outp, nrm2, c2)

            def emit_finals_chunk(qh, nrm2, c2, on_scalar=False):
                for dt4 in range(4):
                    fin = ps.tile([128, 512], F32, tag="small", bufs=2, name="fin")
                    nc.tensor.matmul(
                        fin,
                        lhsT=wo_sb[:, dt4 * 128 : (dt4 + 1) * 128],
                        rhs=nrm2[:, c2 * 512 : (c2 + 1) * 512],
                        start=True,
                        stop=True,
                    )
                    fsb = sbB.tile([128, 512], BF16, tag="fsb", bufs=3, name="fsb")
                    if on_scalar:
                        # tail only: the exp stream is done, ACT engine idle
                        nc.scalar.activation(
                            out=fsb, in_=fin,
                            func=mybir.ActivationFunctionType.Copy,
                        )
                    else:
                        nc.vector.tensor_copy(out=fsb, in_=fin)
                    # contiguous pout block: (qh, c2, dt4) -> [128, 512]
                    blk = (qh * 2 + c2) * 4 + dt4
                    nc.sync.dma_start(out=pout_d[blk], in_=fsb)

            def vproj_steps():
                VT_sb = vtmp.tile([128, L], F32)
                # all on the SP queue: the ACT engine is mid-exp-stream here
                xv_b = x_blocks("v", xvT_d, split=False)

                # remaining q projection chunks (qh=1 halves) first: their x
                # blocks land before xv does.
                def _q23(ch):
                    emit_proj(
                        "q23", xq_b, wq_sb, 0,
                        qm_sb if with_bias else None, evict_qk(QE), chunks=(ch,),
                    )

                def _chunk(ch):
                    sl = slice(ch * 512, (ch + 1) * 512)
                    psp = ps.tile([128, 512], F32, tag="small", bufs=2, name="psp")
                    for t in range(4):
                        nc.tensor.matmul(
                            psp,
                            lhsT=wv_sb[:, t, :],
                            rhs=xv_b[ch][t],
                            start=(t == 0),
                            stop=(t == 3 and not with_bias),
                        )
                    if with_bias:
                        nc.tensor.matmul(
                            psp,
                            lhsT=wb_sb[0:1, 2 * DH2 : 3 * DH2],
                            rhs=ones_sb[0:1, sl],
                            start=False,
                            stop=True,
                        )
                    nc.vector.tensor_copy(out=VT_sb[:, sl], in_=psp)

                def _tp(kt):
                    tp = ps.tile([128, 128], F32, tag="small", bufs=2, name="tp")
                    nc.tensor.transpose(tp, VT_sb[:, kt * 128 : (kt + 1) * 128], ident)
                    for h in range(HPC):
                        nc.vector.tensor_scalar_mul(
                            out=Vp[h][:, kt, 0:DH],
                            in0=tp[:, h * DH : (h + 1) * DH],
                            scalar1=coef_sb[:, kt : kt + 1],
                        )

                for ch in range(2, 4):
                    yield lambda ch=ch: _q23(ch)
                for ch in range(4):
                    yield lambda ch=ch: _chunk(ch)
                for kt in range(NKT):
                    yield lambda kt=kt: _tp(kt)

            # ---- emission: software-pipelined over 4 attention units ----
            emit_proj(
                "k", xk_b, wk_sb, 1, ones_sb if with_bias else None, evict_qk(KE)
            )
            emit_proj(
                "q", xq_b, wq_sb, 0, qm_sb if with_bias else None, evict_qk(QE),
                chunks=(0, 1),
            )

            units = [(0, 0), (0, 1), (1, 0), (1, 1)]
            pts = {}
            outps = {}
            nrm2s = {
                0: sbB.tile([DH2, QH], BF16, tag="nrm", bufs=2, name="nrm2_0"),
                1: sbB.tile([DH2, QH], BF16, tag="nrm", bufs=2, name="nrm2_1"),
            }
            # unit 0 scores interleaved with the v projection/transpose
            pts[0] = ptp.tile([128, NKT, QH], ATT_DT, tag="pt", name="pt0")
            interleave(b1_steps(0, 0, pts[0]), vproj_steps())
            for i in range(1, 4):
                qh, h = units[i]
                pqh, ph = units[i - 1]
                pts[i] = ptp.tile([128, NKT, QH], ATT_DT, tag="pt", name=f"pt{i}")
                outps[i - 1] = ps.tile([65, QH], F32, tag="outp", bufs=1, name="outp")
                interleave(
                    b1_steps(qh, h, pts[i]),
                    b2_steps(pqh, ph, pts[i - 1], outps[i - 1]),
                )
                emit_norm(ph, outps[i - 1], nrm2s[pqh])
                if i == 2:
                    for c2 in range(QH // 512):
                        emit_finals_chunk(0, nrm2s[0], c2)
            # ---- tail: unit 3 b2 chunk-major, norm/finals pipelined ----
            # tag "st" reuses a score-PSUM buffer (free once b1 is done), so
            # the tail does not wait for unit 2's norm to release "outp".
            outp3 = ps.tile([65, QH], F32, tag="st", bufs=2, name="outp3")
            for c2 in range(QH // 512):
                for kt in range(NKT):
                    nc.tensor.matmul(
                        outp3[:, c2 * 512 : (c2 + 1) * 512],
                        lhsT=Vp[1][:, kt, :],
                        rhs=pts[3][:, kt, c2 * 512 : (c2 + 1) * 512],
                        start=(kt == 0),
                        stop=(kt == NKT - 1),
                    )
                emit_norm_chunk(1, outp3, nrm2s[1], c2)
            for c2 in range(QH // 512):
                emit_finals_chunk(1, nrm2s[1], c2, on_scalar=True)

    nc.compile()
    return nc


_CACHE = {}


def _get_nc(with_bias: bool):
    key = ("nc", with_bias)
    if key not in _CACHE:
        _CACHE[key] = build_nc(with_bias)
    return _CACHE[key]


def kernel(q, k, v, text_mask, audio_mask, n_head, wq, bq, wk, bk, wv, bv, wo, bo):
    global LAST_RESULT
    import ml_dtypes

    bf16 = ml_dtypes.bfloat16

    q = np.asarray(q, np.float32)
    k = np.asarray(k, np.float32)
    v = np.asarray(v, np.float32)
    text_mask = np.asarray(text_mask, np.float32)
    audio_mask = np.asarray(audio_mask, np.float32)
    wq = np.asarray(wq, np.float32)
    wk = np.asarray(wk, np.float32)
    wv = np.asarray(wv, np.float32)
    wo = np.asarray(wo, np.float32)
    bq = np.asarray(bq, np.float32)
    bk = np.asarray(bk, np.float32)
    bv = np.asarray(bv, np.float32)
    bo = np.asarray(bo, np.float32)
    assert int(n_head) == H

    with_bias = bool(np.any(bq) or np.any(bk) or np.any(bv))

    pad = np.concatenate([text_mask, audio_mask], axis=1)  # [B, L]
    qm = (pad != 0).astype(np.float32)
    tl = text_mask.sum(1)
    al = audio_mask.sum(1)
    tot = tl + al
    coef = np.concatenate(
        [
            text_mask * (tot / (2.0 * tl))[:, None],
            audio_mask * (tot / (2.0 * al))[:, None],
        ],
        axis=1,
    ).astype(np.float32)
    kbmc = (NEG * (1.0 - qm) - C_LN).astype(np.float32)
    ones_row = np.ones((L,), np.float32)
    cln_row = np.full((L,), C_LN, np.float32)

    def cb(a):
        return np.ascontiguousarray(np.asarray(a, np.float32).astype(bf16))

    def cc(a):
        return np.ascontiguousarray(a, dtype=np.float32)

    def wlayout(w):
        # [DM, DH2] -> SBUF layout [p, t*DH2]: w_sb[p, t, m] = w[t*128+p, m]
        return w.reshape(4, 128, DH2).transpose(1, 0, 2).reshape(128, 4 * DH2)

    in_maps = []
    for core in range(NCORES):
        b, hp = divmod(core, NCORES // B)
        cols = slice(hp * DH2, (hp + 1) * DH2)
        m = {
            "xqT": cb((q[b] * qm[b][:, None]).T),
            "xkT": cb(k[b].T),
            "xvT": cb(v[b].T),
            "wqs": cb(wlayout(wq.T[:, cols])),
            "wks": cb(wlayout(wk.T[:, cols] / 8.0)),
            "wvs": cb(wlayout(wv.T[:, cols])),
            "wos": cb(wo.T[cols, :]),
            "aux": cc(np.stack([qm[b], kbmc[b], ones_row, cln_row])),
            # coef in SBUF layout [p, kt]: coef_sb[p, kt] = coef[kt*128+p]
            "coef": cc(coef[b].reshape(NKT, 128).T),
        }
        if with_bias:
            m["wbias"] = cb(
                np.concatenate(
                    [bq[cols], bk[cols] / 8.0, bv[cols], np.zeros(DH2, np.float32)]
                )
            ).reshape(1, 4 * DH2)
        in_maps.append(m)

    res = run_bass_kernel_spmd(
        _get_nc(with_bias), in_maps, core_ids=list(range(NCORES)), trace=TRACE
    )
    LAST_RESULT = res

    def unblock(arr):
        # [16,128,512] blocks (qh,c2,dt4) -> poutT [DM, L]
        return (
            arr.reshape(2, 2, 4, 128, 512)
            .transpose(2, 3, 0, 1, 4)
            .reshape(DM, L)
        )

    out = np.zeros((B, L, DM), np.float32)
    npc = NCORES // B
    for b in range(B):
        acc = res.results[b * npc]["poutT"].astype(np.float32)
        for hp in range(1, npc):
            acc = acc + res.results[b * npc + hp]["poutT"].astype(np.float32)
        out[b] = unblock(acc).T + bo[None, :]
    return out


# revision 37
# speedup vs baseline: 1.0690x; 1.0690x over previous
"""Trainium2 Bass kernel for CustomMultiHeadAttention (B=2, L=2048, D=512, H=8).

Sharding: 8 cores = 2 batches x 4 head-pairs. Each core computes, for its
batch b and its 2 heads, the partial output (O_h @ Wo_h summed over its
heads), transposed: poutT [512, 2048]. Host sums the 4 partials per batch,
transposes, and adds bo.

Device-side math per core (all masking folded into matmul contractions):
  Qh = (q[b]*qm) @ WqT_cols + qm*bq_cols          (masked q rows -> exactly 0)
  Kh = k[b] @ WkT_cols/8 + bk_cols/8
  E[k,q] = Kh.Qh + (kb[k]-c)*qm[q] + c            via 2 extra contraction rows
           (kb = -1e4 for masked keys, c = ln(1/2048))
    -> unmasked q: E = s + kb  (masked keys underflow to 0 in exp)
    -> masked q:   E = c       (exp = 1/2048 uniform; denom = 1)
  PT = exp(E)   [k, q] layout
  outT = [Vp | 1]^T @ PT   (Vp = coef * Vh; ones column yields denom row)
  O = outT[0:64] / outT[64]
  poutT[d, q] += Wo[:, d] . O2[:, q]   (both heads packed, contraction 128)

Perf notes vs the first version:
  - x and weights travel as bf16 (halves input DMA), x in [128,512] blocks
    chunk-major so the first projection matmul starts ~2us in.
  - PE warm-up matmuls at t=0 ride the DVFS ramp (0.65->2.4GHz after ~3us
    of continuous busy).
  - Output projection packs both heads into contraction-128 matmuls.
  - reciprocal_approx_fast (~5x faster than reciprocal); norm + finals are
    chunked at 512 cols so the tail pipelines instead of serializing.
  - poutT returns as bf16 (halves output DMA).
"""

import math
import os

os.environ.setdefault("MYCRO_LOCAL_CACHE", "1")

import numpy as np

import concourse.bass as bass
import concourse.tile as tile
from concourse import bacc
from concourse import mybir
from concourse.bass_utils import run_bass_kernel_spmd
from concourse.masks import make_identity

B = 2
L = 2048
DM = 512
H = 8
DH = 64
NCORES = 8
HPC = 2           # heads per core
DH2 = HPC * DH    # 128
NKT = L // 128    # 16 k tiles
QH = 1024         # q chunk for attention phase
NQH = L // QH     # 2
C_LN = -math.log(L)
NEG = -10000.0

F32 = mybir.dt.float32
F32R = mybir.dt.float32r
BF16 = mybir.dt.bfloat16
F8 = mybir.dt.float8e4

ATT_DT = BF16     # exp output (PT), V'
QK_DT = F32R      # QE/KE on-chip operands (full-rate, ~tf32 rounding)
X_DT = BF16       # x / w wire + projection matmul dtype
NKT2 = NKT // 2   # key tile PAIRS for DoubleRow PV

TRACE = False
LAST_RESULT = None

AUX_QM, AUX_KBMC, AUX_ONES, AUX_CLN = 0, 1, 2, 3


def build_nc(with_bias: bool):
    nc = bacc.Bacc(None, target_bir_lowering=False)

    xqT_d = nc.declare_dram_parameter("xqT", [DM, L], X_DT, isOutput=False)
    xkT_d = nc.declare_dram_parameter("xkT", [DM, L], X_DT, isOutput=False)
    xvT_d = nc.declare_dram_parameter("xvT", [DM, L], X_DT, isOutput=False)
    # weights pre-arranged host-side to the SBUF layout [p, t, m] so the DMA
    # is one contiguous 1KB-per-partition transfer (the on-the-fly rearrange
    # generated 256B packets and crawled at ~37 GB/s).
    wqs_d = nc.declare_dram_parameter("wqs", [128, 4 * DH2], X_DT, isOutput=False)
    wks_d = nc.declare_dram_parameter("wks", [128, 4 * DH2], X_DT, isOutput=False)
    wvs_d = nc.declare_dram_parameter("wvs", [128, 4 * DH2], X_DT, isOutput=False)
    if with_bias:
        wbias_d = nc.declare_dram_parameter(
            "wbias", [1, 4 * DH2], X_DT, isOutput=False
        )
    wos_d = nc.declare_dram_parameter("wos", [DH2, DM], BF16, isOutput=False)
    aux_d = nc.declare_dram_parameter("aux", [4, L], QK_DT, isOutput=False)
    coef_d = nc.declare_dram_parameter("coef", [128, NKT], F32, isOutput=False)
    # poutT stored as 16 contiguous [128, 512] blocks, index (qh*2+c2)*4+dt4;
    # host reassembles. Contiguous blocks DMA at full rate (strided rows of
    # the [DM, L] layout only reached ~78 GB/s).
    pout_d = nc.declare_dram_parameter("poutT", [16, 128, 512], BF16, isOutput=True)

    with tile.TileContext(nc) as tc:
        with (
            tc.tile_pool(name="const", bufs=1) as const,
            tc.tile_pool(name="qek", bufs=1) as qek,
            tc.tile_pool(name="xin", bufs=1) as xin,
            tc.tile_pool(name="vtmp", bufs=1) as vtmp,
            tc.tile_pool(name="ptp", bufs=2) as ptp,
            tc.tile_pool(name="sbB", bufs=1) as sbB,
            tc.tile_pool(name="ps", bufs=1, space="PSUM") as ps,
        ):
            # ---- constants: all small tensors stream before the x blocks ----
            ident = const.tile([128, 128], F32)
            make_identity(nc, ident)
            wk_sb = const.tile([128, 4, DH2], X_DT)
            nc.sync.dma_start(out=wk_sb, in_=wks_d[:, :])
            wq_sb = const.tile([128, 4, DH2], X_DT)
            nc.scalar.dma_start(out=wq_sb, in_=wqs_d[:, :])
            wv_sb = const.tile([128, 4, DH2], X_DT)
            nc.scalar.dma_start(out=wv_sb, in_=wvs_d[:, :])
            wo_sb = const.tile([DH2, DM], BF16)
            nc.scalar.dma_start(out=wo_sb, in_=wos_d[:, :])
            coef_sb = const.tile([128, NKT], F32)
            nc.scalar.dma_start(out=coef_sb, in_=coef_d[:, :])
            if with_bias:
                wb_sb = const.tile([1, 4 * DH2], X_DT)
                nc.sync.dma_start(out=wb_sb, in_=wbias_d[:, :])
                qm_sb = const.tile([1, L], X_DT)
                ones_sb = const.tile([1, L], X_DT)
                nc.vector.memset(ones_sb, 1.0)

            # ---- x input blocks: [128, 512], chunk-major so chunk 0 of a
            # projection is computable after 4 small DMAs. Blocks alternate
            # between the two hardware DMA queues (SP + ACT engines) for
            # ~2x aggregate HBM read bandwidth; the ACT engine is idle
            # during the projection phase so its queue is free.
            def x_blocks(pname, xdram, split=True):
                blocks = []
                for ch in range(4):
                    col = []
                    for t in range(4):
                        xt = xin.tile(
                            [128, 512], X_DT, tag="xin", bufs=48,
                            name=f"x{pname}{t}_{ch}",
                        )
                        eng = nc.scalar if (split and t >= 2) else nc.sync
                        eng.dma_start(
                            out=xt,
                            in_=xdram[
                                t * 128 : (t + 1) * 128,
                                ch * 512 : (ch + 1) * 512,
                            ],
                        )
                        col.append(xt)
                    blocks.append(col)
                return blocks

            # ---- persistent per-head operands ----
            QE = [qek.tile([66, L], QK_DT, name=f"QE{h}") for h in range(HPC)]
            KE = [qek.tile([66, L], QK_DT, name=f"KE{h}") for h in range(HPC)]
            Vp = [
                qek.tile([128, NKT, DH + 1], ATT_DT, name=f"Vp{h}") for h in range(HPC)
            ]
            # mask/bias rows of the extended operands (DMA direct from host aux)
            for h in range(HPC):
                nc.sync.dma_start(
                    out=QE[h][64:65, :], in_=aux_d[AUX_QM : AUX_QM + 1, :]
                )
                nc.sync.dma_start(
                    out=QE[h][65:66, :], in_=aux_d[AUX_ONES : AUX_ONES + 1, :]
                )
                nc.scalar.dma_start(
                    out=KE[h][64:65, :], in_=aux_d[AUX_KBMC : AUX_KBMC + 1, :]
                )
                nc.scalar.dma_start(
                    out=KE[h][65:66, :], in_=aux_d[AUX_CLN : AUX_CLN + 1, :]
                )
                nc.vector.memset(Vp[h][:, :, DH : DH + 1], 1.0)

            # k streams first (b1 needs all of K), then the q chunks that
            # feed qh=0 (cols 0-1023), then the rest of q during attention.
            # All x blocks ride the SP queue; the ACT queue only carries the
            # small consts (x blocks there would stall the exp stream).
            xk_b = x_blocks("k", xkT_d, split=False)
            xq_b = x_blocks("q", xqT_d, split=False)
            if with_bias:
                nc.sync.dma_start(out=qm_sb, in_=aux_d[AUX_QM : AUX_QM + 1, :])

            def emit_proj(pname, xb, w_sb, brow, brhs, evict, chunks=(0, 1, 2, 3)):
                for ch in chunks:
                    sl = slice(ch * 512, (ch + 1) * 512)
                    psp = ps.tile([128, 512], F32, tag="small", bufs=2, name="psp")
                    for t in range(4):
                        nc.tensor.matmul(
                            psp,
                            lhsT=w_sb[:, t, :],
                            rhs=xb[ch][t],
                            start=(t == 0),
                            stop=(t == 3 and not with_bias),
                        )
                    if with_bias:
                        nc.tensor.matmul(
                            psp,
                            lhsT=wb_sb[0:1, brow * DH2 : (brow + 1) * DH2],
                            rhs=brhs[0:1, sl],
                            start=False,
                            stop=True,
                        )
                    evict(psp, sl)

            def evict_qk(dst):
                def _e(psp, sl):
                    for h in range(HPC):
                        nc.vector.tensor_copy(
                            out=dst[h][0:DH, sl], in_=psp[h * DH : (h + 1) * DH, :]
                        )

                return _e

            def b1_step(qh, h, pt, kt):
                st = ps.tile([128, QH], F32, tag="st", bufs=2, name="st")
                for c2 in range(QH // 512):
                    nc.tensor.matmul(
                        st[:, c2 * 512 : (c2 + 1) * 512],
                        lhsT=KE[h][0:66, kt * 128 : (kt + 1) * 128],
                        rhs=QE[h][
                            0:66, qh * QH + c2 * 512 : qh * QH + (c2 + 1) * 512
                        ],
                        start=True,
                        stop=True,
                    )
                nc.scalar.activation(
                    out=pt[:, kt, :], in_=st, func=mybir.ActivationFunctionType.Exp
                )

            def b1_steps(qh, h, pt):
                for kt in range(NKT):
                    yield lambda kt=kt: b1_step(qh, h, pt, kt)

            def b2_steps(qh, h, pt, outp):
                for kt in range(NKT):
                    def _s(kt=kt):
                        for c2 in range(QH // 512):
                            nc.tensor.matmul(
                                outp[:, c2 * 512 : (c2 + 1) * 512],
                                lhsT=Vp[h][:, kt, :],
                                rhs=pt[:, kt, c2 * 512 : (c2 + 1) * 512],
                                start=(kt == 0),
                                stop=(kt == NKT - 1),
                            )
                    yield _s

            def interleave(*gens):
                gens = [iter(g) for g in gens if g is not None]
                while gens:
                    nxt = []
                    for g in gens:
                        try:
                            next(g)()
                        except StopIteration:
                            continue
                        nxt.append(g)
                    gens = nxt

            from concourse.alu_op_type import AluOpType

            def emit_norm_chunk(h, outp, nrm2, c2):
                sl = slice(c2 * 512, (c2 + 1) * 512)
                den = sbB.tile([1, 512], F32, tag="den", bufs=2, name="den")
                nc.vector.tensor_copy(out=den, in_=outp[DH : DH + 1, sl])
                rcp = sbB.tile([1, 512], F32, tag="rcp", bufs=2, name="rcp")
                nc.vector.reciprocal_approx_fast(out=rcp, in_=den)
                rbc = sbB.tile([DH, 512], F32, tag="rbc", bufs=2, name="rbc")
                nc.gpsimd.partition_broadcast(rbc, rcp[0:1, :], channels=DH)
                # nrm2 = (outp * 1.0) * rbc  — fused PSUM read + scale
                nc.vector.scalar_tensor_tensor(
                    out=nrm2[h * DH : (h + 1) * DH, sl],
                    in0=outp[0:DH, sl],
                    scalar=1.0,
                    in1=rbc,
                    op0=AluOpType.mult,
                    op1=AluOpType.mult,
                )

            def emit_norm(h, outp, nrm2):
                for c2 in range(QH // 512):
                    emit_norm_chunk(h, outp, nrm2, c2)

            def emit_norm_chunk3(h, o3c, nrm2, c2):
                # like emit_norm_chunk, but the PSUM input is already the
                # [65, 512] chunk for columns c2
                sl = slice(c2 * 512, (c2 + 1) * 512)
                den = sbB.tile([1, 512], F32, tag="den", bufs=2, name="den")
                nc.vector.tensor_copy(out=den, in_=o3c[DH : DH + 1, :])
                rcp = sbB.tile([1, 512], F32, tag="rcp", bufs=2, name="rcp")
                nc.vector.reciprocal_approx_fast(out=rcp, in_=den)
                rbc = sbB.tile([DH, 512], F32, tag="rbc", bufs=2, name="rbc")
                nc.gpsimd.partition_broadcast(rbc, rcp[0:1, :], channels=DH)
                nc.vector.scalar_tensor_tensor(
                    out=nrm2[h * DH : (h + 1) * DH, sl],
                    in0=o3c[0:DH, :],
                    scalar=1.0,
                    in1=rbc,
                    op0=AluOpType.mult,
                    op1=AluOpType.mult,
                )

            def emit_finals_chunk(qh, nrm2, c2, on_scalar=False):
                for dt4 in range(4):
                    fin = ps.tile([128, 512], F32, tag="small", bufs=2, name="fin")
                    nc.tensor.matmul(
                        fin,
                        lhsT=wo_sb[:, dt4 * 128 : (dt4 + 1) * 128],
                        rhs=nrm2[:, c2 * 512 : (c2 + 1) * 512],
                        start=True,
                        stop=True,
                    )
                    fsb = sbB.tile([128, 512], BF16, tag="fsb", bufs=3, name="fsb")
                    if on_scalar:
                        # tail only: the exp stream is done, ACT engine idle
                        nc.scalar.activation(
                            out=fsb, in_=fin,
                            func=mybir.ActivationFunctionType.Copy,
                        )
                    else:
                        nc.vector.tensor_copy(out=fsb, in_=fin)
                    # contiguous pout block: (qh, c2, dt4) -> [128, 512]
                    blk = (qh * 2 + c2) * 4 + dt4
                    nc.sync.dma_start(out=pout_d[blk], in_=fsb)

            def vproj_steps():
                VT_sb = vtmp.tile([128, L], F32)
                # all on the SP queue: the ACT engine is mid-exp-stream here
                xv_b = x_blocks("v", xvT_d, split=False)

                # remaining q projection chunks (qh=1 halves) first: their x
                # blocks land before xv does.
                def _q23(ch):
                    emit_proj(
                        "q23", xq_b, wq_sb, 0,
                        qm_sb if with_bias else None, evict_qk(QE), chunks=(ch,),
                    )

                def _chunk(ch):
                    sl = slice(ch * 512, (ch + 1) * 512)
                    psp = ps.tile([128, 512], F32, tag="small", bufs=2, name="psp")
                    for t in range(4):
                        nc.tensor.matmul(
                            psp,
                            lhsT=wv_sb[:, t, :],
                            rhs=xv_b[ch][t],
                            start=(t == 0),
                            stop=(t == 3 and not with_bias),
                        )
                    if with_bias:
                        nc.tensor.matmul(
                            psp,
                            lhsT=wb_sb[0:1, 2 * DH2 : 3 * DH2],
                            rhs=ones_sb[0:1, sl],
                            start=False,
                            stop=True,
                        )
                    nc.vector.tensor_copy(out=VT_sb[:, sl], in_=psp)

                def _tp(kt):
                    tp = ps.tile([128, 128], F32, tag="small", bufs=2, name="tp")
                    nc.tensor.transpose(tp, VT_sb[:, kt * 128 : (kt + 1) * 128], ident)
                    for h in range(HPC):
                        nc.vector.tensor_scalar_mul(
                            out=Vp[h][:, kt, 0:DH],
                            in0=tp[:, h * DH : (h + 1) * DH],
                            scalar1=coef_sb[:, kt : kt + 1],
                        )

                for ch in range(2, 4):
                    yield lambda ch=ch: _q23(ch)
                for ch in range(4):
                    yield lambda ch=ch: _chunk(ch)
                for kt in range(NKT):
                    yield lambda kt=kt: _tp(kt)

            # ---- emission: software-pipelined over 4 attention units ----
            emit_proj(
                "k", xk_b, wk_sb, 1, ones_sb if with_bias else None, evict_qk(KE)
            )
            emit_proj(
                "q", xq_b, wq_sb, 0, qm_sb if with_bias else None, evict_qk(QE),
                chunks=(0, 1),
            )

            units = [(0, 0), (0, 1), (1, 0), (1, 1)]
            pts = {}
            outps = {}
            nrm2s = {
                0: sbB.tile([DH2, QH], BF16, tag="nrm", bufs=2, name="nrm2_0"),
                1: sbB.tile([DH2, QH], BF16, tag="nrm", bufs=2, name="nrm2_1"),
            }
            # unit 0 scores interleaved with the v projection/transpose
            pts[0] = ptp.tile([128, NKT, QH], ATT_DT, tag="pt", name="pt0")
            interleave(b1_steps(0, 0, pts[0]), vproj_steps())
            for i in range(1, 4):
                qh, h = units[i]
                pqh, ph = units[i - 1]
                pts[i] = ptp.tile([128, NKT, QH], ATT_DT, tag="pt", name=f"pt{i}")
                outps[i - 1] = ps.tile([65, QH], F32, tag="outp", bufs=1, name="outp")
                interleave(
                    b1_steps(qh, h, pts[i]),
                    b2_steps(pqh, ph, pts[i - 1], outps[i - 1]),
                )
                emit_norm(ph, outps[i - 1], nrm2s[pqh])
                if i == 2:
                    for c2 in range(QH // 512):
                        emit_finals_chunk(0, nrm2s[0], c2)
            # ---- tail: unit 3 b2 chunk-major, norm/finals pipelined ----
            # per-chunk [65,512] tiles on the "small" tag: those buffers free
            # early (tag "st" buffers are pinned until the LAST exp reads
            # them, which stalled the tail ~3.4us; "outp" waits on unit 2's
            # norm chain).
            for c2 in range(QH // 512):
                o3c = ps.tile([65, 512], F32, tag="small", bufs=2, name=f"o3c{c2}")
                for kt in range(NKT):
                    nc.tensor.matmul(
                        o3c,
                        lhsT=Vp[1][:, kt, :],
                        rhs=pts[3][:, kt, c2 * 512 : (c2 + 1) * 512],
                        start=(kt == 0),
                        stop=(kt == NKT - 1),
                    )
                emit_norm_chunk3(1, o3c, nrm2s[1], c2)
            for c2 in range(QH // 512):
                emit_finals_chunk(1, nrm2s[1], c2, on_scalar=True)

    nc.compile()
    return nc


_CACHE = {}


def _get_nc(with_bias: bool):
    key = ("nc", with_bias)
    if key not in _CACHE:
        _CACHE[key] = build_nc(with_bias)
    return _CACHE[key]


def kernel(q, k, v, text_mask, audio_mask, n_head, wq, bq, wk, bk, wv, bv, wo, bo):
    global LAST_RESULT
    import ml_dtypes

    bf16 = ml_dtypes.bfloat16

    q = np.asarray(q, np.float32)
    k = np.asarray(k, np.float32)
    v = np.asarray(v, np.float32)
    text_mask = np.asarray(text_mask, np.float32)
    audio_mask = np.asarray(audio_mask, np.float32)
    wq = np.asarray(wq, np.float32)
    wk = np.asarray(wk, np.float32)
    wv = np.asarray(wv, np.float32)
    wo = np.asarray(wo, np.float32)
    bq = np.asarray(bq, np.float32)
    bk = np.asarray(bk, np.float32)
    bv = np.asarray(bv, np.float32)
    bo = np.asarray(bo, np.float32)
    assert int(n_head) == H

    with_bias = bool(np.any(bq) or np.any(bk) or np.any(bv))

    pad = np.concatenate([text_mask, audio_mask], axis=1)  # [B, L]
    qm = (pad != 0).astype(np.float32)
    tl = text_mask.sum(1)
    al = audio_mask.sum(1)
    tot = tl + al
    coef = np.concatenate(
        [
            text_mask * (tot / (2.0 * tl))[:, None],
            audio_mask * (tot / (2.0 * al))[:, None],
        ],
        axis=1,
    ).astype(np.float32)
    kbmc = (NEG * (1.0 - qm) - C_LN).astype(np.float32)
    ones_row = np.ones((L,), np.float32)
    cln_row = np.full((L,), C_LN, np.float32)

    def cb(a):
        return np.ascontiguousarray(np.asarray(a, np.float32).astype(bf16))

    def cc(a):
        return np.ascontiguousarray(a, dtype=np.float32)

    def wlayout(w):
        # [DM, DH2] -> SBUF layout [p, t*DH2]: w_sb[p, t, m] = w[t*128+p, m]
        return w.reshape(4, 128, DH2).transpose(1, 0, 2).reshape(128, 4 * DH2)

    in_maps = []
    for core in range(NCORES):
        b, hp = divmod(core, NCORES // B)
        cols = slice(hp * DH2, (hp + 1) * DH2)
        m = {
            "xqT": cb((q[b] * qm[b][:, None]).T),
            "xkT": cb(k[b].T),
            "xvT": cb(v[b].T),
            "wqs": cb(wlayout(wq.T[:, cols])),
            "wks": cb(wlayout(wk.T[:, cols] / 8.0)),
            "wvs": cb(wlayout(wv.T[:, cols])),
            "wos": cb(wo.T[cols, :]),
            "aux": cc(np.stack([qm[b], kbmc[b], ones_row, cln_row])),
            # coef in SBUF layout [p, kt]: coef_sb[p, kt] = coef[kt*128+p]
            "coef": cc(coef[b].reshape(NKT, 128).T),
        }
        if with_bias:
            m["wbias"] = cb(
                np.concatenate(
                    [bq[cols], bk[cols] / 8.0, bv[cols], np.zeros(DH2, np.float32)]
                )
            ).reshape(1, 4 * DH2)
        in_maps.append(m)

    res = run_bass_kernel_spmd(
        _get_nc(with_bias), in_maps, core_ids=list(range(NCORES)), trace=TRACE
    )
    LAST_RESULT = res

    def unblock(arr):
        # [16,128,512] blocks (qh,c2,dt4) -> poutT [DM, L]
        return (
            arr.reshape(2, 2, 4, 128, 512)
            .transpose(2, 3, 0, 1, 4)
            .reshape(DM, L)
        )

    out = np.zeros((B, L, DM), np.float32)
    npc = NCORES // B
    for b in range(B):
        acc = res.results[b * npc]["poutT"].astype(np.float32)
        for hp in range(1, npc):
            acc = acc + res.results[b * npc + hp]["poutT"].astype(np.float32)
        out[b] = unblock(acc).T + bo[None, :]
    return out
